# revision 18
# baseline (speedup 1.0000x reference)
"""Trainium2 Bass kernel for AuxiliaryGovernedAttention.

Math (see reference):
  q       = hidden @ W_q.T / sqrt(64)                    [B,S,D]
  scores  = q @ aux_keys.T + log(reliability + 1e-10)    [B,S,NS]
  attn    = softmax(scores, -1)
  aux_out = attn @ aux_values                            [B,S,H]
  avg_w   = mean_h(primary_attention_weights)            [B,S,S]
  entropy = -sum(avg_w * log(avg_w + 1e-10), -1)         [B,S]
  gate    = sigmoid(w1*entropy + b); veto <0.5 -> 0; >2.0 -> min(gate, 0.8)
  out     = primary_attention_output + gate * aux_out

Sharding: flatten (B,S) -> 4096 query rows; core c owns rows
[c*512, (c+1)*512) (batch c//4, seq block c%4). All small tensors are
replicated; no collectives.

The dominant cost is streaming primary_attention_weights, which only
feeds an entropy reduction, so it is shipped as fp8e4 (host-side
scale-by-2048 keeps values in e4m3 mid-range; the scale is folded back
into the Ln and gate constants), cutting the stream from 134 MB/core
(f32) to 33.5 MB/core.  The stream arrives as 16 x 2MB 8-head group
DMAs (host-grouped contiguous; the first group is split per-head so
TensorE can start ~5us sooner) - few, large transfers keep the
sequencers' per-dma_start issue cost and DMA-semaphore reuse churn off
the critical path.  The 32-head sum runs on TensorE as fp8
identity-matmul PSUM accumulation (1 col/cycle, weight reloads hidden
by FWL); ACT computes Ln straight out of PSUM and DVE does the
x*ln(x) reduce.  The aux output path is precomputed per block before
its head-sum, so each block's tail is gate -> fused (aux*gate)+pao ->
store; dummy [128,1] activations pre-load the Ln/Sigmoid tables so no
1.3us ACT table load sits on the tail.  hidden / W_q ride fp8
(host-prescaled, compensated when draining the PSUM), pao is bf16,
and the output is stored bf16 and upcast on host.  Per-core HBM
traffic ~44 MB.
"""

import sys
from contextlib import ExitStack

import ml_dtypes
import numpy as np

sys.path.insert(0, "/opt/trn_rl_repo")

import concourse.mybir as mybir
import concourse.tile as tile
from concourse import bacc
from concourse.bass_utils import run_bass_kernel_spmd

F32 = mybir.dt.float32
BF16 = mybir.dt.bfloat16
FP8 = mybir.dt.float8e4
AF = mybir.ActivationFunctionType
ALU = mybir.AluOpType

B, S, H, NH, NS, D = 2, 2048, 4096, 32, 100, 64
NCORES = 8
ROWS = (B * S) // NCORES    # 512 query rows per core
BLK = 128                   # queries per block (partition dim)
NBLK = ROWS // BLK          # 4 blocks per core
KT = H // 128               # 32 k-tiles for the q projection
HCH = 512                   # aux-output free chunk (one PSUM bank)
NHCH = H // HCH             # 8 chunks
SCH = 512                   # entropy free chunk (one PSUM bank)
NSCH = S // SCH             # 4 chunks
GH = 8                      # heads per paw DMA group
NGRP = NH // GH             # 4 groups per block
PSCALE = 2048.0             # host-side fp8 pre-scale for paw
RSC = NH * PSCALE           # paw psum-domain normalizer (65536)
WSC = 128.0                 # host-side fp8 pre-scale for W_q

_GRAPH_CACHE = {}


def build_graph():
    nc = bacc.Bacc()
    hst_d = nc.declare_dram_parameter("hst", [128, KT * ROWS], FP8, isOutput=False)
    pao_d = nc.declare_dram_parameter("pao", [ROWS, H], BF16, isOutput=False)
    paw_d = nc.declare_dram_parameter(
        "paw", [NBLK * NGRP, BLK, GH * S], FP8, isOutput=False
    )
    wqt_d = nc.declare_dram_parameter("wqt", [128, KT * D], FP8, isOutput=False)
    akt_d = nc.declare_dram_parameter("akt", [D, NS], BF16, isOutput=False)
    av_d = nc.declare_dram_parameter("av", [NS, H], BF16, isOutput=False)
    cst_d = nc.declare_dram_parameter("cst", [128, 4 + NS], F32, isOutput=False)
    idt_d = nc.declare_dram_parameter("idt", [128, 128], F32, isOutput=False)
    id8_d = nc.declare_dram_parameter("id8", [128, 128], FP8, isOutput=False)
    out_d = nc.declare_dram_parameter("out", [ROWS, H], BF16, isOutput=True)

    with ExitStack() as ctx:
        tc = ctx.enter_context(tile.TileContext(nc))
        const_p = ctx.enter_context(tc.tile_pool(name="const", bufs=1))
        paw0_p = ctx.enter_context(tc.tile_pool(name="paw0", bufs=GH))
        paw_p = ctx.enter_context(tc.tile_pool(name="paw", bufs=4))
        hst_p = ctx.enter_context(tc.tile_pool(name="hst", bufs=1))
        ln_p = ctx.enter_context(tc.tile_pool(name="ln", bufs=2))
        prod_p = ctx.enter_context(tc.tile_pool(name="prod", bufs=2))
        pao_p = ctx.enter_context(tc.tile_pool(name="pao", bufs=2))
        out_p = ctx.enter_context(tc.tile_pool(name="out", bufs=2))
        aux_p = ctx.enter_context(tc.tile_pool(name="aux", bufs=3))
        small_p = ctx.enter_context(tc.tile_pool(name="small", bufs=2))
        acc_ps = ctx.enter_context(tc.tile_pool(name="acc_ps", bufs=5, space="PSUM"))
        ax_ps = ctx.enter_context(tc.tile_pool(name="ax_ps", bufs=2, space="PSUM"))
        mi_ps = ctx.enter_context(tc.tile_pool(name="mi_ps", bufs=1, space="PSUM"))

        # ---- one-time constants (ACT HWDGE ring); id8 + hst first so the
        # head-sum and q-projection unblock as early as possible
        id8 = const_p.tile([128, 128], FP8, tag="id8")
        nc.scalar.dma_start(out=id8[:], in_=id8_d[:])
        hst_t = hst_p.tile([128, KT * ROWS], FP8, tag="hst")
        nc.scalar.dma_start(out=hst_t[:], in_=hst_d[:])
        wqt = const_p.tile([128, KT * D], FP8, tag="wqt")
        nc.scalar.dma_start(out=wqt[:], in_=wqt_d[:])
        cst = const_p.tile([128, 4 + NS], F32, tag="cst")
        nc.scalar.dma_start(out=cst[:], in_=cst_d[:])
        akt = const_p.tile([D, NS], BF16, tag="akt")
        nc.scalar.dma_start(out=akt[:], in_=akt_d[:])
        av = const_p.tile([NS, H], BF16, tag="av")
        nc.scalar.dma_start(out=av[:], in_=av_d[:])
        ident = const_p.tile([128, 128], F32, tag="ident")
        nc.scalar.dma_start(out=ident[:], in_=idt_d[:])

        # ---- the paw fp8 stream (SP HWDGE ring, alone).  Block 0's first
        # group arrives as 8 per-head 256KB DMAs (early PE start); the
        # remaining 15 groups as whole 2MB transfers.
        pw0_tiles = []
        for h in range(GH):
            pw = paw0_p.tile([BLK, S], FP8, tag="pw0", name=f"pw0_{h}")
            nc.sync.dma_start(out=pw[:], in_=paw_d[0, :, h * S : (h + 1) * S])
            pw0_tiles.append(pw)
        pw_tiles = {}
        for b in range(NBLK):
            for g in range(NGRP):
                if b == 0 and g == 0:
                    continue
                pw = paw_p.tile([BLK, GH * S], FP8, tag="pw", name=f"pw{b}_{g}")
                nc.sync.dma_start(out=pw[:], in_=paw_d[b * NGRP + g])
                pw_tiles[(b, g)] = pw

        # residual loads (SWDGE ring, with the stores)
        pao_tiles = []
        for b in range(NBLK):
            pao_t = pao_p.tile([BLK, H], BF16, tag="pao", name=f"pao{b}")
            nc.gpsimd.dma_start(
                out=pao_t[:], in_=pao_d[b * BLK : (b + 1) * BLK, :]
            )
            pao_tiles.append(pao_t)

        def rhs_slice(b, h, c):
            g, hh = divmod(h, GH)
            if b == 0 and g == 0:
                return pw0_tiles[hh][:, c * SCH : (c + 1) * SCH]
            base = hh * S + c * SCH
            return pw_tiles[(b, g)][:, base : base + SCH]

        def head_sum(b):
            """fp8 identity-matmul accumulation: 4 interleaved 32-matmul
            PSUM groups (one per 512-col chunk)."""
            acc = [
                acc_ps.tile([BLK, SCH], F32, tag="acc", name=f"acc{b}_{c}")
                for c in range(NSCH)
            ]
            for h in range(NH):
                for c in range(NSCH):
                    nc.tensor.matmul(
                        acc[c][:],
                        lhsT=id8[:],
                        rhs=rhs_slice(b, h, c),
                        start=(h == 0),
                        stop=(h == NH - 1),
                    )
            return acc

        def entropy_gate(b, acc):
            """Ln straight from PSUM, x*ln(x) reduce, sigmoid gate+veto."""
            rr = small_p.tile([BLK, NSCH], F32, tag="rr", name=f"rr{b}")
            ln_last = None
            for c in range(NSCH):
                ln_t = ln_p.tile([BLK, SCH], BF16, tag="ln", name=f"ln{b}_{c}")
                nc.scalar.activation(
                    ln_t[:], acc[c][:], AF.Ln, bias=cst[:, 2:3], scale=1.0 / RSC
                )
                prod = prod_p.tile([BLK, SCH], BF16, tag="prod", name=f"pr{b}_{c}")
                nc.vector.tensor_mul(prod[:], acc[c][:], ln_t[:])
                nc.vector.reduce_sum(
                    rr[:, c : c + 1], prod[:], axis=mybir.AxisListType.X
                )
                ln_last = ln_t
            # dummy [128,1] Sigmoid forced after the last Ln: pre-loads the
            # sigmoid ACT table while DVE finishes the reduce
            scr = small_p.tile([BLK, 1], F32, tag="scr", name=f"scrs{b}")
            nc.scalar.activation(scr[:], ln_last[:, :1], AF.Sigmoid)
            r_t = small_p.tile([BLK, 1], F32, tag="r", name=f"r{b}")
            nc.vector.reduce_sum(r_t[:], rr[:], axis=mybir.AxisListType.X)
            # gate = sigmoid(w1*ent + bias);  ent = -r'/RSC
            g0 = small_p.tile([BLK, 1], F32, tag="g0", name=f"g0{b}")
            nc.scalar.activation(
                g0[:], r_t[:], AF.Sigmoid, bias=cst[:, 1:2], scale=cst[:, 0:1]
            )
            # dummy [128,1] Ln forced after the sigmoid: pre-loads the Ln
            # table for the next block off the critical path
            if b < NBLK - 1:
                scr2 = small_p.tile([BLK, 1], F32, tag="scr2", name=f"scrl{b}")
                nc.scalar.activation(scr2[:], g0[:], AF.Ln, bias=cst[:, 2:3])
            # veto: ent<0.5 (r'>-.5RSC) -> 0 ; ent>2.0 (r'<-2RSC) -> min(g,.8)
            mlo = small_p.tile([BLK, 1], F32, tag="mlo", name=f"ml{b}")
            nc.vector.tensor_scalar(mlo[:], r_t[:], -0.5 * RSC, None, op0=ALU.is_le)
            mhi = small_p.tile([BLK, 1], F32, tag="mhi", name=f"mh{b}")
            nc.vector.tensor_scalar(mhi[:], r_t[:], -2.0 * RSC, None, op0=ALU.is_lt)
            exc = small_p.tile([BLK, 1], F32, tag="exc", name=f"ex{b}")
            nc.vector.tensor_scalar(
                exc[:], g0[:], 0.8, 0.0, op0=ALU.subtract, op1=ALU.max
            )
            nc.vector.tensor_mul(exc[:], exc[:], mhi[:])
            nc.vector.tensor_sub(g0[:], g0[:], exc[:])
            nc.vector.tensor_mul(g0[:], g0[:], mlo[:])
            return g0

        def aux_pre(b, inv4, pt_b):
            """softmax-normalized aux output for the block, gate NOT yet
            applied: axp = (attn @ av) / sum  -> bf16 SBUF."""
            axp = aux_p.tile([BLK, H], BF16, tag="axp", name=f"axp{b}")
            for j in range(NHCH):
                ax = ax_ps.tile([BLK, HCH], F32, tag="ax", name=f"ax{b}_{j}")
                nc.tensor.matmul(
                    ax[:], lhsT=pt_b[:], rhs=av[:, j * HCH : (j + 1) * HCH]
                )
                nc.scalar.activation(
                    axp[:, j * HCH : (j + 1) * HCH],
                    ax[:],
                    AF.Copy,
                    scale=inv4[:, b : b + 1],
                )
            return axp

        def finish(b, g0, axp):
            """out = (axp * gate) + pao fused on DVE, in 2 halves, each
            stored as soon as it is ready."""
            out_t = out_p.tile([BLK, H], BF16, tag="out", name=f"out{b}")
            r0 = b * BLK
            HH = H // 2
            for half in range(2):
                sl = slice(half * HH, (half + 1) * HH)
                nc.vector.scalar_tensor_tensor(
                    out=out_t[:, sl],
                    in0=axp[:, sl],
                    scalar=g0[:],
                    in1=pao_tiles[b][:, sl],
                    op0=ALU.mult,
                    op1=ALU.add,
                )
                nc.gpsimd.dma_start(
                    out=out_d[r0 : r0 + BLK, sl], in_=out_t[:, sl]
                )

        # ---- q projection: qT[64, 512] (fp8, host-prescaled by WSC) ----
        qt_full = mi_ps.tile([BLK, HCH], F32, tag="mi", name="qt_psum")
        qt_psum = qt_full[:D, :ROWS]
        for k in range(KT):
            nc.tensor.matmul(
                qt_psum[:],
                lhsT=wqt[:, k * D : (k + 1) * D],
                rhs=hst_t[:, k * ROWS : (k + 1) * ROWS],
                start=(k == 0),
                stop=(k == KT - 1),
            )
        qt_sb = const_p.tile([D, ROWS], BF16, tag="qt_sb")
        nc.scalar.activation(qt_sb[:], qt_psum[:], AF.Copy, scale=1.0 / WSC)

        # ---- scores / softmax numerator / attn transpose, all blocks ----
        inv4 = const_p.tile([128, NBLK], F32, tag="inv4")
        pt_all = []
        for b in range(NBLK):
            r0 = b * BLK
            sc_full = mi_ps.tile([BLK, HCH], F32, tag="mi", name=f"sc_psum{b}")
            sc_psum = sc_full[:, :NS]
            nc.tensor.matmul(sc_psum[:], lhsT=qt_sb[:, r0 : r0 + BLK], rhs=akt[:])
            sc_sb = small_p.tile([BLK, NS], F32, tag="sc_sb", name=f"scb{b}")
            nc.vector.tensor_add(sc_sb[:], sc_psum[:], cst[:, 4 : 4 + NS])
            p_t = small_p.tile([BLK, NS], F32, tag="p", name=f"p{b}")
            ssum = small_p.tile([BLK, 1], F32, tag="ssum", name=f"ss{b}")
            nc.scalar.activation(
                p_t[:], sc_sb[:], AF.Exp, bias=cst[:, 3:4], accum_out=ssum[:]
            )
            nc.vector.reciprocal(inv4[:, b : b + 1], ssum[:])
            pt_full = mi_ps.tile([BLK, HCH], F32, tag="mi", name=f"pt_psum{b}")
            pt_psum = pt_full[:NS, :BLK]
            nc.tensor.transpose(pt_psum[:], p_t[:], ident[:])
            ptb = const_p.tile([NS, BLK], BF16, tag=f"pt{b}")
            nc.scalar.copy(ptb[:], pt_psum[:])
            pt_all.append(ptb)

        acc0 = head_sum(0)
        axp0 = aux_pre(0, inv4, pt_all[0])
        axp1 = aux_pre(1, inv4, pt_all[1])
        g0_0 = entropy_gate(0, acc0)
        finish(0, g0_0, axp0)

        acc1 = head_sum(1)
        axp2 = aux_pre(2, inv4, pt_all[2])
        axp3 = aux_pre(3, inv4, pt_all[3])
        g0_1 = entropy_gate(1, acc1)
        finish(1, g0_1, axp1)

        acc2 = head_sum(2)
        g0_2 = entropy_gate(2, acc2)
        finish(2, g0_2, axp2)

        acc3 = head_sum(3)
        g0_3 = entropy_gate(3, acc3)
        finish(3, g0_3, axp3)

    nc.compile()
    return nc


def _get_graph():
    key = "g"
    if key not in _GRAPH_CACHE:
        _GRAPH_CACHE[key] = build_graph()
    return _GRAPH_CACHE[key]


def _make_in_maps(inputs):
    hs = np.asarray(inputs["hidden_states"], dtype=np.float32).reshape(B * S, H)
    pao = np.asarray(inputs["primary_attention_output"], dtype=np.float32).reshape(
        B * S, H
    )
    paw = np.asarray(inputs["primary_attention_weights"], dtype=np.float32)
    rel = np.asarray(inputs["reliability"], dtype=np.float32)
    wq = np.asarray(inputs["W_q"], dtype=np.float32)
    ak = np.asarray(inputs["aux_keys"], dtype=np.float32)
    av = np.asarray(inputs["aux_values"], dtype=np.float32)
    w1 = float(np.asarray(inputs["gate_w1"]))
    gb = float(np.asarray(inputs["gate_bias"]))

    bf = ml_dtypes.bfloat16
    f8 = ml_dtypes.float8_e4m3
    # W_q.T with the 1/sqrt(64) folded in and a fp8-range prescale of WSC
    # (taken back out when draining the q-projection PSUM), laid out as 32
    # stacked [128, 64] k-tiles along the free axis.
    wqt = (
        (wq * (0.125 * WSC)).T
        .reshape(KT, 128, D)
        .transpose(1, 0, 2)
        .reshape(128, KT * D)
    )
    wqt = np.ascontiguousarray(wqt).astype(f8)
    akt = np.ascontiguousarray(ak.T).astype(bf)
    avc = np.ascontiguousarray(av).astype(bf)

    cst = np.zeros((128, 4 + NS), dtype=np.float32)
    cst[:, 0] = -w1 / RSC    # Sigmoid scale (ent = -r'/RSC)
    cst[:, 1] = gb           # Sigmoid bias
    cst[:, 2] = 1e-10        # Ln bias
    cst[:, 3] = 0.0          # Exp bias (scores)
    cst[:, 4:] = np.log(rel + 1e-10)[None, :]

    paw8 = (paw * PSCALE).astype(f8)

    in_maps = []
    for c in range(NCORES):
        bb = c // (NCORES // B)
        s0 = (c % (NCORES // B)) * ROWS
        rows = slice(c * ROWS, (c + 1) * ROWS)
        hst = (
            np.ascontiguousarray(hs[rows].T)
            .astype(f8)
            .reshape(KT, 128, ROWS)
            .transpose(1, 0, 2)
            .reshape(128, KT * ROWS)
        )
        # paw -> [block, group, row, head, s] contiguous per (block, group)
        pw = (
            paw8[bb, :, s0 : s0 + ROWS, :]
            .reshape(NGRP, GH, NBLK, BLK, S)
            .transpose(2, 0, 3, 1, 4)
            .reshape(NBLK * NGRP, BLK, GH * S)
        )
        in_maps.append(
            {
                "hst": np.ascontiguousarray(hst),
                "pao": np.ascontiguousarray(pao[rows]).astype(bf),
                "paw": np.ascontiguousarray(pw),
                "wqt": wqt,
                "akt": akt,
                "av": avc,
                "cst": cst,
                "idt": np.eye(128, dtype=np.float32),
                "id8": np.eye(128).astype(f8),
            }
        )
    return in_maps


def kernel(**inputs) -> np.ndarray:
    nc = _get_graph()
    in_maps = _make_in_maps(inputs)
    res = run_bass_kernel_spmd(nc, in_maps, list(range(NCORES)))
    out = np.concatenate([res.results[i]["out"] for i in range(NCORES)], axis=0)
    return np.ascontiguousarray(out.reshape(B, S, H).astype(np.float32))


def kernel_traced(inputs, **kw):
    """test-harness entry: returns (output, BassKernelResults)."""
    nc = _get_graph()
    in_maps = _make_in_maps(inputs)
    res = run_bass_kernel_spmd(nc, in_maps, list(range(NCORES)), trace=True, **kw)
    out = np.concatenate([res.results[i]["out"] for i in range(NCORES)], axis=0)
    return np.ascontiguousarray(out.reshape(B, S, H).astype(np.float32)), res


# revision 19
# speedup vs baseline: 1.0703x; 1.0703x over previous
"""Trainium2 Bass kernel for AuxiliaryGovernedAttention.

Math (see reference):
  q       = hidden @ W_q.T / sqrt(64)                    [B,S,D]
  scores  = q @ aux_keys.T + log(reliability + 1e-10)    [B,S,NS]
  attn    = softmax(scores, -1)
  aux_out = attn @ aux_values                            [B,S,H]
  avg_w   = mean_h(primary_attention_weights)            [B,S,S]
  entropy = -sum(avg_w * log(avg_w + 1e-10), -1)         [B,S]
  gate    = sigmoid(w1*entropy + b); veto <0.5 -> 0; >2.0 -> min(gate, 0.8)
  out     = primary_attention_output + gate * aux_out

Sharding: flatten (B,S) -> 4096 query rows; core c owns rows
[c*512, (c+1)*512) (batch c//4, seq block c%4). All small tensors are
replicated; no collectives.

The dominant cost is streaming primary_attention_weights, which only
feeds an entropy reduction, so it is shipped as fp8e4 (host-side
scale-by-2048 keeps values in e4m3 mid-range; the scale is folded back
into the Ln and gate constants), cutting the stream from 134 MB/core
(f32) to 33.5 MB/core.  The stream arrives as 16 x 2MB 8-head group
DMAs (host-grouped contiguous; the first group is split per-head so
TensorE can start ~5us sooner) - few, large transfers keep the
sequencers' per-dma_start issue cost and DMA-semaphore reuse churn off
the critical path.  The 32-head sum runs on TensorE as fp8
identity-matmul PSUM accumulation (1 col/cycle, weight reloads hidden
by FWL); ACT computes Ln straight out of PSUM and DVE does the
x*ln(x) reduce.  The aux output path is precomputed per block before
its head-sum, so each block's tail is gate -> fused (aux*gate)+pao ->
store; dummy [128,1] activations pre-load the Ln/Sigmoid tables so no
1.3us ACT table load sits on the tail.  hidden / W_q ride fp8
(host-prescaled, compensated when draining the PSUM), pao is bf16,
and the output is stored bf16 and upcast on host.  Per-core HBM
traffic ~44 MB.
"""

import sys
from contextlib import ExitStack

import ml_dtypes
import numpy as np

sys.path.insert(0, "/opt/trn_rl_repo")

import concourse.mybir as mybir
import concourse.tile as tile
from concourse import bacc
from concourse.bass_utils import run_bass_kernel_spmd

F32 = mybir.dt.float32
BF16 = mybir.dt.bfloat16
FP8 = mybir.dt.float8e4
AF = mybir.ActivationFunctionType
ALU = mybir.AluOpType

B, S, H, NH, NS, D = 2, 2048, 4096, 32, 100, 64
NCORES = 8
ROWS = (B * S) // NCORES    # 512 query rows per core
BLK = 128                   # queries per block (partition dim)
NBLK = ROWS // BLK          # 4 blocks per core
KT = H // 128               # 32 k-tiles for the q projection
HCH = 512                   # aux-output free chunk (one PSUM bank)
NHCH = H // HCH             # 8 chunks
SCH = 512                   # entropy free chunk (one PSUM bank)
NSCH = S // SCH             # 4 chunks
GH = 8                      # heads per paw DMA group
NGRP = NH // GH             # 4 groups per block
PSCALE = 2048.0             # host-side fp8 pre-scale for paw
RSC = NH * PSCALE           # paw psum-domain normalizer (65536)
WSC = 128.0                 # host-side fp8 pre-scale for W_q

_GRAPH_CACHE = {}


def build_graph():
    nc = bacc.Bacc()
    hst_d = nc.declare_dram_parameter("hst", [128, KT * ROWS], FP8, isOutput=False)
    pao_d = nc.declare_dram_parameter("pao", [ROWS, H], BF16, isOutput=False)
    paw_d = nc.declare_dram_parameter(
        "paw", [NBLK * NGRP, BLK, GH * S], FP8, isOutput=False
    )
    wqt_d = nc.declare_dram_parameter("wqt", [128, KT * D], FP8, isOutput=False)
    akt_d = nc.declare_dram_parameter("akt", [D, NS], BF16, isOutput=False)
    av_d = nc.declare_dram_parameter("av", [NS, H], BF16, isOutput=False)
    cst_d = nc.declare_dram_parameter("cst", [128, 4 + NS], F32, isOutput=False)
    idt_d = nc.declare_dram_parameter("idt", [128, 128], F32, isOutput=False)
    id8_d = nc.declare_dram_parameter("id8", [128, 2 * 128], FP8, isOutput=False)
    out_d = nc.declare_dram_parameter("out", [ROWS, H], BF16, isOutput=True)

    with ExitStack() as ctx:
        tc = ctx.enter_context(tile.TileContext(nc))
        const_p = ctx.enter_context(tc.tile_pool(name="const", bufs=1))
        paw0_p = ctx.enter_context(tc.tile_pool(name="paw0", bufs=GH))
        paw_p = ctx.enter_context(tc.tile_pool(name="paw", bufs=5))
        hst_p = ctx.enter_context(tc.tile_pool(name="hst", bufs=1))
        ln_p = ctx.enter_context(tc.tile_pool(name="ln", bufs=2))
        prod_p = ctx.enter_context(tc.tile_pool(name="prod", bufs=2))
        pao_p = ctx.enter_context(tc.tile_pool(name="pao", bufs=1))
        out_p = ctx.enter_context(tc.tile_pool(name="out", bufs=2))
        aux_p = ctx.enter_context(tc.tile_pool(name="aux", bufs=3))
        small_p = ctx.enter_context(tc.tile_pool(name="small", bufs=2))
        acc_ps = ctx.enter_context(tc.tile_pool(name="acc_ps", bufs=5, space="PSUM"))
        ax_ps = ctx.enter_context(tc.tile_pool(name="ax_ps", bufs=2, space="PSUM"))
        mi_ps = ctx.enter_context(tc.tile_pool(name="mi_ps", bufs=1, space="PSUM"))

        # ---- one-time constants (ACT HWDGE ring); id8 + hst first so the
        # head-sum and q-projection unblock as early as possible
        id8 = const_p.tile([128, 2, 128], FP8, tag="id8")
        id8p = const_p.tile([128, 128], FP8, tag="id8p")
        nc.scalar.dma_start(out=id8[:], in_=id8_d[:])
        nc.scalar.dma_start(out=id8p[:], in_=id8_d[:, :128])
        hst_t = hst_p.tile([128, KT * ROWS], FP8, tag="hst")
        nc.scalar.dma_start(out=hst_t[:], in_=hst_d[:])
        wqt = const_p.tile([128, KT * D], FP8, tag="wqt")
        nc.scalar.dma_start(out=wqt[:], in_=wqt_d[:])
        cst = const_p.tile([128, 4 + NS], F32, tag="cst")
        nc.scalar.dma_start(out=cst[:], in_=cst_d[:])
        akt = const_p.tile([D, NS], BF16, tag="akt")
        nc.scalar.dma_start(out=akt[:], in_=akt_d[:])
        av = const_p.tile([NS, H], BF16, tag="av")
        nc.scalar.dma_start(out=av[:], in_=av_d[:])
        ident = const_p.tile([128, 128], F32, tag="ident")
        nc.scalar.dma_start(out=ident[:], in_=idt_d[:])

        # ---- the paw fp8 stream (SP HWDGE ring, alone).  Block 0's first
        # group arrives as 8 per-head 256KB DMAs (early PE start); the
        # remaining 15 groups as whole 2MB transfers.
        pw0_tiles = []
        for h in range(GH):
            pw = paw0_p.tile([BLK, S], FP8, tag="pw0", name=f"pw0_{h}")
            nc.sync.dma_start(out=pw[:], in_=paw_d[0, :, h * S : (h + 1) * S])
            pw0_tiles.append(pw)
        pw_tiles = {}
        for b in range(NBLK):
            for g in range(NGRP):
                if b == 0 and g == 0:
                    continue
                pw = paw_p.tile([BLK, GH, S], FP8, tag="pw", name=f"pw{b}_{g}")
                nc.sync.dma_start(out=pw[:], in_=paw_d[b * NGRP + g])
                pw_tiles[(b, g)] = pw

        # residual loads (SWDGE ring, with the stores)
        pao_tiles = []
        for b in range(NBLK):
            pao_t = pao_p.tile([BLK, H], BF16, tag="pao", name=f"pao{b}")
            nc.gpsimd.dma_start(
                out=pao_t[:], in_=pao_d[b * BLK : (b + 1) * BLK, :]
            )
            pao_tiles.append(pao_t)

        def head_sum(b):
            """head accumulation into 4 interleaved PSUM chunk groups:
            DoubleRow fp8 pair-matmuls on the 2MB group tiles; plain
            per-head matmuls on block 0's split first group."""
            acc = [
                acc_ps.tile([BLK, SCH], F32, tag="acc", name=f"acc{b}_{c}")
                for c in range(NSCH)
            ]
            for g in range(NGRP):
                first, last = (g == 0), (g == NGRP - 1)
                if b == 0 and g == 0:
                    for h in range(GH):
                        for c in range(NSCH):
                            nc.tensor.matmul(
                                acc[c][:],
                                lhsT=id8p[:],
                                rhs=pw0_tiles[h][:, c * SCH : (c + 1) * SCH],
                                start=(h == 0),
                                stop=False,
                            )
                    continue
                pw = pw_tiles[(b, g)]
                for j in range(GH // 2):
                    for c in range(NSCH):
                        nc.tensor.matmul(
                            acc[c][:],
                            lhsT=id8[:],
                            rhs=pw[:, 2 * j : 2 * j + 2, c * SCH : (c + 1) * SCH],
                            start=(first and j == 0),
                            stop=(last and j == GH // 2 - 1),
                            perf_mode=mybir.MatmulPerfMode.DoubleRow,
                        )
            return acc

        def entropy_gate(b, acc):
            """Ln straight from PSUM, x*ln(x) reduce, sigmoid gate+veto."""
            rr = small_p.tile([BLK, NSCH], F32, tag="rr", name=f"rr{b}")
            ln_last = None
            for c in range(NSCH):
                ln_t = ln_p.tile([BLK, SCH], BF16, tag="ln", name=f"ln{b}_{c}")
                nc.scalar.activation(
                    ln_t[:], acc[c][:], AF.Ln, bias=cst[:, 2:3], scale=1.0 / RSC
                )
                prod = prod_p.tile([BLK, SCH], BF16, tag="prod", name=f"pr{b}_{c}")
                nc.vector.tensor_mul(prod[:], acc[c][:], ln_t[:])
                nc.vector.reduce_sum(
                    rr[:, c : c + 1], prod[:], axis=mybir.AxisListType.X
                )
                ln_last = ln_t
            # dummy [128,1] Sigmoid forced after the last Ln: pre-loads the
            # sigmoid ACT table while DVE finishes the reduce
            scr = small_p.tile([BLK, 1], F32, tag="scr", name=f"scrs{b}")
            nc.scalar.activation(scr[:], ln_last[:, :1], AF.Sigmoid)
            r_t = small_p.tile([BLK, 1], F32, tag="r", name=f"r{b}")
            nc.vector.reduce_sum(r_t[:], rr[:], axis=mybir.AxisListType.X)
            # gate = sigmoid(w1*ent + bias);  ent = -r'/RSC
            g0 = small_p.tile([BLK, 1], F32, tag="g0", name=f"g0{b}")
            nc.scalar.activation(
                g0[:], r_t[:], AF.Sigmoid, bias=cst[:, 1:2], scale=cst[:, 0:1]
            )
            # dummy [128,1] Ln forced after the sigmoid: pre-loads the Ln
            # table for the next block off the critical path
            if b < NBLK - 1:
                scr2 = small_p.tile([BLK, 1], F32, tag="scr2", name=f"scrl{b}")
                nc.scalar.activation(scr2[:], g0[:], AF.Ln, bias=cst[:, 2:3])
            # veto: ent<0.5 (r'>-.5RSC) -> 0 ; ent>2.0 (r'<-2RSC) -> min(g,.8)
            mlo = small_p.tile([BLK, 1], F32, tag="mlo", name=f"ml{b}")
            nc.vector.tensor_scalar(mlo[:], r_t[:], -0.5 * RSC, None, op0=ALU.is_le)
            mhi = small_p.tile([BLK, 1], F32, tag="mhi", name=f"mh{b}")
            nc.vector.tensor_scalar(mhi[:], r_t[:], -2.0 * RSC, None, op0=ALU.is_lt)
            exc = small_p.tile([BLK, 1], F32, tag="exc", name=f"ex{b}")
            nc.vector.tensor_scalar(
                exc[:], g0[:], 0.8, 0.0, op0=ALU.subtract, op1=ALU.max
            )
            nc.vector.tensor_mul(exc[:], exc[:], mhi[:])
            nc.vector.tensor_sub(g0[:], g0[:], exc[:])
            nc.vector.tensor_mul(g0[:], g0[:], mlo[:])
            return g0

        def aux_pre(b, inv4, pt_b):
            """softmax-normalized aux output for the block, gate NOT yet
            applied: axp = (attn @ av) / sum  -> bf16 SBUF."""
            axp = aux_p.tile([BLK, H], BF16, tag="axp", name=f"axp{b}")
            for j in range(NHCH):
                ax = ax_ps.tile([BLK, HCH], F32, tag="ax", name=f"ax{b}_{j}")
                nc.tensor.matmul(
                    ax[:], lhsT=pt_b[:], rhs=av[:, j * HCH : (j + 1) * HCH]
                )
                nc.scalar.activation(
                    axp[:, j * HCH : (j + 1) * HCH],
                    ax[:],
                    AF.Copy,
                    scale=inv4[:, b : b + 1],
                )
            return axp

        def finish(b, g0, axp):
            """out = (axp * gate) + pao fused on DVE, in 2 halves, each
            stored as soon as it is ready."""
            out_t = out_p.tile([BLK, H], BF16, tag="out", name=f"out{b}")
            r0 = b * BLK
            HH = H // 2
            for half in range(2):
                sl = slice(half * HH, (half + 1) * HH)
                nc.vector.scalar_tensor_tensor(
                    out=out_t[:, sl],
                    in0=axp[:, sl],
                    scalar=g0[:],
                    in1=pao_tiles[b][:, sl],
                    op0=ALU.mult,
                    op1=ALU.add,
                )
                nc.gpsimd.dma_start(
                    out=out_d[r0 : r0 + BLK, sl], in_=out_t[:, sl]
                )

        # ---- q projection: qT[64, 512] (fp8, host-prescaled by WSC) ----
        qt_full = mi_ps.tile([BLK, HCH], F32, tag="mi", name="qt_psum")
        qt_psum = qt_full[:D, :ROWS]
        for k in range(KT):
            nc.tensor.matmul(
                qt_psum[:],
                lhsT=wqt[:, k * D : (k + 1) * D],
                rhs=hst_t[:, k * ROWS : (k + 1) * ROWS],
                start=(k == 0),
                stop=(k == KT - 1),
            )
        qt_sb = const_p.tile([D, ROWS], BF16, tag="qt_sb")
        nc.scalar.activation(qt_sb[:], qt_psum[:], AF.Copy, scale=1.0 / WSC)

        # ---- scores / softmax numerator / attn transpose, all blocks ----
        inv4 = const_p.tile([128, NBLK], F32, tag="inv4")
        pt_all = []
        for b in range(NBLK):
            r0 = b * BLK
            sc_full = mi_ps.tile([BLK, HCH], F32, tag="mi", name=f"sc_psum{b}")
            sc_psum = sc_full[:, :NS]
            nc.tensor.matmul(sc_psum[:], lhsT=qt_sb[:, r0 : r0 + BLK], rhs=akt[:])
            sc_sb = small_p.tile([BLK, NS], F32, tag="sc_sb", name=f"scb{b}")
            nc.vector.tensor_add(sc_sb[:], sc_psum[:], cst[:, 4 : 4 + NS])
            p_t = small_p.tile([BLK, NS], F32, tag="p", name=f"p{b}")
            ssum = small_p.tile([BLK, 1], F32, tag="ssum", name=f"ss{b}")
            nc.scalar.activation(
                p_t[:], sc_sb[:], AF.Exp, bias=cst[:, 3:4], accum_out=ssum[:]
            )
            nc.vector.reciprocal(inv4[:, b : b + 1], ssum[:])
            pt_full = mi_ps.tile([BLK, HCH], F32, tag="mi", name=f"pt_psum{b}")
            pt_psum = pt_full[:NS, :BLK]
            nc.tensor.transpose(pt_psum[:], p_t[:], ident[:])
            ptb = const_p.tile([NS, BLK], BF16, tag=f"pt{b}")
            nc.scalar.copy(ptb[:], pt_psum[:])
            pt_all.append(ptb)

        acc0 = head_sum(0)
        axp0 = aux_pre(0, inv4, pt_all[0])
        axp1 = aux_pre(1, inv4, pt_all[1])
        g0_0 = entropy_gate(0, acc0)
        finish(0, g0_0, axp0)

        acc1 = head_sum(1)
        axp2 = aux_pre(2, inv4, pt_all[2])
        axp3 = aux_pre(3, inv4, pt_all[3])
        g0_1 = entropy_gate(1, acc1)
        finish(1, g0_1, axp1)

        acc2 = head_sum(2)
        g0_2 = entropy_gate(2, acc2)
        finish(2, g0_2, axp2)

        acc3 = head_sum(3)
        g0_3 = entropy_gate(3, acc3)
        finish(3, g0_3, axp3)

    nc.compile()
    return nc


def _get_graph():
    key = "g"
    if key not in _GRAPH_CACHE:
        _GRAPH_CACHE[key] = build_graph()
    return _GRAPH_CACHE[key]


def _make_in_maps(inputs):
    hs = np.asarray(inputs["hidden_states"], dtype=np.float32).reshape(B * S, H)
    pao = np.asarray(inputs["primary_attention_output"], dtype=np.float32).reshape(
        B * S, H
    )
    paw = np.asarray(inputs["primary_attention_weights"], dtype=np.float32)
    rel = np.asarray(inputs["reliability"], dtype=np.float32)
    wq = np.asarray(inputs["W_q"], dtype=np.float32)
    ak = np.asarray(inputs["aux_keys"], dtype=np.float32)
    av = np.asarray(inputs["aux_values"], dtype=np.float32)
    w1 = float(np.asarray(inputs["gate_w1"]))
    gb = float(np.asarray(inputs["gate_bias"]))

    bf = ml_dtypes.bfloat16
    f8 = ml_dtypes.float8_e4m3
    # W_q.T with the 1/sqrt(64) folded in and a fp8-range prescale of WSC
    # (taken back out when draining the q-projection PSUM), laid out as 32
    # stacked [128, 64] k-tiles along the free axis.
    wqt = (
        (wq * (0.125 * WSC)).T
        .reshape(KT, 128, D)
        .transpose(1, 0, 2)
        .reshape(128, KT * D)
    )
    wqt = np.ascontiguousarray(wqt).astype(f8)
    akt = np.ascontiguousarray(ak.T).astype(bf)
    avc = np.ascontiguousarray(av).astype(bf)

    cst = np.zeros((128, 4 + NS), dtype=np.float32)
    cst[:, 0] = -w1 / RSC    # Sigmoid scale (ent = -r'/RSC)
    cst[:, 1] = gb           # Sigmoid bias
    cst[:, 2] = 1e-10        # Ln bias
    cst[:, 3] = 0.0          # Exp bias (scores)
    cst[:, 4:] = np.log(rel + 1e-10)[None, :]

    paw8 = (paw * PSCALE).astype(f8)
    eye8 = np.eye(128).astype(f8)
    id8h = np.ascontiguousarray(np.hstack([eye8, eye8]))

    in_maps = []
    for c in range(NCORES):
        bb = c // (NCORES // B)
        s0 = (c % (NCORES // B)) * ROWS
        rows = slice(c * ROWS, (c + 1) * ROWS)
        hst = (
            np.ascontiguousarray(hs[rows].T)
            .astype(f8)
            .reshape(KT, 128, ROWS)
            .transpose(1, 0, 2)
            .reshape(128, KT * ROWS)
        )
        # paw -> [block, group, row, head, s] contiguous per (block, group)
        pw = (
            paw8[bb, :, s0 : s0 + ROWS, :]
            .reshape(NGRP, GH, NBLK, BLK, S)
            .transpose(2, 0, 3, 1, 4)
            .reshape(NBLK * NGRP, BLK, GH * S)
        )
        in_maps.append(
            {
                "hst": np.ascontiguousarray(hst),
                "pao": np.ascontiguousarray(pao[rows]).astype(bf),
                "paw": np.ascontiguousarray(pw),
                "wqt": wqt,
                "akt": akt,
                "av": avc,
                "cst": cst,
                "idt": np.eye(128, dtype=np.float32),
                "id8": id8h,
            }
        )
    return in_maps


def kernel(**inputs) -> np.ndarray:
    nc = _get_graph()
    in_maps = _make_in_maps(inputs)
    res = run_bass_kernel_spmd(nc, in_maps, list(range(NCORES)))
    out = np.concatenate([res.results[i]["out"] for i in range(NCORES)], axis=0)
    return np.ascontiguousarray(out.reshape(B, S, H).astype(np.float32))


def kernel_traced(inputs, **kw):
    """test-harness entry: returns (output, BassKernelResults)."""
    nc = _get_graph()
    in_maps = _make_in_maps(inputs)
    res = run_bass_kernel_spmd(nc, in_maps, list(range(NCORES)), trace=True, **kw)
    out = np.concatenate([res.results[i]["out"] for i in range(NCORES)], axis=0)
    return np.ascontiguousarray(out.reshape(B, S, H).astype(np.float32)), res


# revision 20
# speedup vs baseline: 1.1781x; 1.1007x over previous
"""Trainium2 Bass kernel for AuxiliaryGovernedAttention.

Math (see reference):
  q       = hidden @ W_q.T / sqrt(64)                    [B,S,D]
  scores  = q @ aux_keys.T + log(reliability + 1e-10)    [B,S,NS]
  attn    = softmax(scores, -1)
  aux_out = attn @ aux_values                            [B,S,H]
  avg_w   = mean_h(primary_attention_weights)            [B,S,S]
  entropy = -sum(avg_w * log(avg_w + 1e-10), -1)         [B,S]
  gate    = sigmoid(w1*entropy + b); veto <0.5 -> 0; >2.0 -> min(gate, 0.8)
  out     = primary_attention_output + gate * aux_out

Sharding: flatten (B,S) -> 4096 query rows; core c owns rows
[c*512, (c+1)*512) (batch c//4, seq block c%4). All small tensors are
replicated; no collectives.

The dominant cost is streaming primary_attention_weights, which only
feeds an entropy reduction, so it is shipped as fp8e4 (host-side
scale-by-2048 keeps values in e4m3 mid-range; the scale is folded back
into the Ln and gate constants), cutting the stream from 134 MB/core
(f32) to 33.5 MB/core.  The stream arrives as 16 x 2MB group DMAs
(8 heads each, host-grouped contiguous) - large transfers keep the SP
sequencer's ~0.7us-per-dma_start issue cost off the critical path.
The 32-head sum runs on TensorE as DoubleRow fp8 pair-matmuls
(identity-pair weights, 2 elem/partition/cycle) accumulating in PSUM;
ACT computes Ln straight out of PSUM and DVE does the x*ln(x) reduce.
The aux output path is precomputed per block before its head-sum, so
each block's tail is gate -> fused (aux*gate)+pao -> store.  hidden /
W_q ride fp8 (host-prescaled, compensated when draining the PSUM),
pao is bf16, and the output is stored bf16 and upcast on host.
Per-core HBM traffic ~44 MB.
"""

import sys
from contextlib import ExitStack

import ml_dtypes
import numpy as np

sys.path.insert(0, "/opt/trn_rl_repo")

import concourse.mybir as mybir
import concourse.tile as tile
from concourse import bacc
from concourse.bass_utils import run_bass_kernel_spmd

F32 = mybir.dt.float32
BF16 = mybir.dt.bfloat16
FP8 = mybir.dt.float8e4
AF = mybir.ActivationFunctionType
ALU = mybir.AluOpType
DR = mybir.MatmulPerfMode.DoubleRow

B, S, H, NH, NS, D = 2, 2048, 4096, 32, 100, 64
NCORES = 8
ROWS = (B * S) // NCORES    # 512 query rows per core
BLK = 128                   # queries per block (partition dim)
NBLK = ROWS // BLK          # 4 blocks per core
KT = H // 128               # 32 k-tiles for the q projection
HCH = 512                   # aux-output free chunk (one PSUM bank)
NHCH = H // HCH             # 8 chunks
SCH = 512                   # entropy free chunk (one PSUM bank)
NSCH = S // SCH             # 4 chunks
GH = 8                      # heads per paw DMA group
NGRP = NH // GH             # 4 groups per block
PSCALE = 2048.0             # host-side fp8 pre-scale for paw
RSC = NH * PSCALE           # paw psum-domain normalizer (65536)
WSC = 128.0                 # host-side fp8 pre-scale for W_q

_GRAPH_CACHE = {}


def build_graph():
    nc = bacc.Bacc()
    hst_d = nc.declare_dram_parameter("hst", [128, KT * ROWS], FP8, isOutput=False)
    pao_d = nc.declare_dram_parameter("pao", [ROWS, H], BF16, isOutput=False)
    paw_d = nc.declare_dram_parameter(
        "paw", [NBLK * NGRP, BLK, GH * S], FP8, isOutput=False
    )
    wqt_d = nc.declare_dram_parameter("wqt", [128, KT * D], FP8, isOutput=False)
    akt_d = nc.declare_dram_parameter("akt", [D, NS], BF16, isOutput=False)
    av_d = nc.declare_dram_parameter("av", [NS, H], BF16, isOutput=False)
    cst_d = nc.declare_dram_parameter("cst", [128, 4 + NS], F32, isOutput=False)
    idt_d = nc.declare_dram_parameter("idt", [128, 128], F32, isOutput=False)
    id8_d = nc.declare_dram_parameter("id8", [128, 2 * 128], FP8, isOutput=False)
    out_d = nc.declare_dram_parameter("out", [ROWS, H], BF16, isOutput=True)

    with ExitStack() as ctx:
        tc = ctx.enter_context(tile.TileContext(nc))
        const_p = ctx.enter_context(tc.tile_pool(name="const", bufs=1))
        paw_p = ctx.enter_context(tc.tile_pool(name="paw", bufs=5))
        hst_p = ctx.enter_context(tc.tile_pool(name="hst", bufs=1))
        ln_p = ctx.enter_context(tc.tile_pool(name="ln", bufs=2))
        prod_p = ctx.enter_context(tc.tile_pool(name="prod", bufs=2))
        pao_p = ctx.enter_context(tc.tile_pool(name="pao", bufs=2))
        out_p = ctx.enter_context(tc.tile_pool(name="out", bufs=2))
        aux_p = ctx.enter_context(tc.tile_pool(name="aux", bufs=4))
        small_p = ctx.enter_context(tc.tile_pool(name="small", bufs=2))
        acc_ps = ctx.enter_context(tc.tile_pool(name="acc_ps", bufs=4, space="PSUM"))
        ax_ps = ctx.enter_context(tc.tile_pool(name="ax_ps", bufs=2, space="PSUM"))
        mi_ps = ctx.enter_context(tc.tile_pool(name="mi_ps", bufs=1, space="PSUM"))

        # ---- one-time constants (ACT HWDGE ring) ----
        id8 = const_p.tile([128, 2, 128], FP8, tag="id8")
        nc.scalar.dma_start(out=id8[:], in_=id8_d[:])
        ident = const_p.tile([128, 128], F32, tag="ident")
        nc.scalar.dma_start(out=ident[:], in_=idt_d[:])
        cst = const_p.tile([128, 4 + NS], F32, tag="cst")
        nc.scalar.dma_start(out=cst[:], in_=cst_d[:])
        akt = const_p.tile([D, NS], BF16, tag="akt")
        nc.scalar.dma_start(out=akt[:], in_=akt_d[:])
        av = const_p.tile([NS, H], BF16, tag="av")
        nc.scalar.dma_start(out=av[:], in_=av_d[:])
        wqt = const_p.tile([128, KT * D], FP8, tag="wqt")
        nc.scalar.dma_start(out=wqt[:], in_=wqt_d[:])
        hst_t = hst_p.tile([128, KT * ROWS], FP8, tag="hst")
        nc.scalar.dma_start(out=hst_t[:], in_=hst_d[:])

        # ---- the paw fp8 stream: 16 x 2MB grouped DMAs (SP HWDGE ring) ----
        pw_tiles = {}
        for b in range(NBLK):
            for g in range(NGRP):
                pw = paw_p.tile([BLK, GH, S], FP8, tag="pw", name=f"pw{b}_{g}")
                nc.sync.dma_start(out=pw[:], in_=paw_d[b * NGRP + g])
                pw_tiles[(b, g)] = pw

        # residual loads (ACT ring)
        pao_tiles = []
        for b in range(NBLK):
            pao_t = pao_p.tile([BLK, H], BF16, tag="pao", name=f"pao{b}")
            nc.scalar.dma_start(
                out=pao_t[:], in_=pao_d[b * BLK : (b + 1) * BLK, :]
            )
            pao_tiles.append(pao_t)

        def head_sum(b):
            """DoubleRow fp8 pair-matmuls: 4 interleaved 16-matmul PSUM
            accumulation groups (one per 512-col chunk)."""
            acc = [
                acc_ps.tile([BLK, SCH], F32, tag="acc", name=f"acc{b}_{c}")
                for c in range(NSCH)
            ]
            for g in range(NGRP):
                pw = pw_tiles[(b, g)]
                for j in range(GH // 2):
                    for c in range(NSCH):
                        nc.tensor.matmul(
                            acc[c][:],
                            lhsT=id8[:],
                            rhs=pw[:, 2 * j : 2 * j + 2, c * SCH : (c + 1) * SCH],
                            start=(g == 0 and j == 0),
                            stop=(g == NGRP - 1 and j == GH // 2 - 1),
                            perf_mode=DR,
                        )
            return acc

        def entropy_gate(b, acc):
            """Ln straight from PSUM, x*ln(x) reduce, sigmoid gate+veto."""
            rr = small_p.tile([BLK, NSCH], F32, tag="rr", name=f"rr{b}")
            for c in range(NSCH):
                ln_t = ln_p.tile([BLK, SCH], BF16, tag="ln", name=f"ln{b}_{c}")
                nc.scalar.activation(
                    ln_t[:], acc[c][:], AF.Ln, bias=cst[:, 2:3], scale=1.0 / RSC
                )
                prod = prod_p.tile([BLK, SCH], BF16, tag="prod", name=f"pr{b}_{c}")
                nc.vector.tensor_mul(prod[:], acc[c][:], ln_t[:])
                nc.vector.reduce_sum(
                    rr[:, c : c + 1], prod[:], axis=mybir.AxisListType.X
                )
            r_t = small_p.tile([BLK, 1], F32, tag="r", name=f"r{b}")
            nc.vector.reduce_sum(r_t[:], rr[:], axis=mybir.AxisListType.X)
            # gate = sigmoid(w1*ent + bias);  ent = -r'/RSC
            g0 = small_p.tile([BLK, 1], F32, tag="g0", name=f"g0{b}")
            nc.scalar.activation(
                g0[:], r_t[:], AF.Sigmoid, bias=cst[:, 1:2], scale=cst[:, 0:1]
            )
            # veto: ent<0.5 (r'>-.5RSC) -> 0 ; ent>2.0 (r'<-2RSC) -> min(g,.8)
            mlo = small_p.tile([BLK, 1], F32, tag="mlo", name=f"ml{b}")
            nc.vector.tensor_scalar(mlo[:], r_t[:], -0.5 * RSC, None, op0=ALU.is_le)
            mhi = small_p.tile([BLK, 1], F32, tag="mhi", name=f"mh{b}")
            nc.vector.tensor_scalar(mhi[:], r_t[:], -2.0 * RSC, None, op0=ALU.is_lt)
            exc = small_p.tile([BLK, 1], F32, tag="exc", name=f"ex{b}")
            nc.vector.tensor_scalar(
                exc[:], g0[:], 0.8, 0.0, op0=ALU.subtract, op1=ALU.max
            )
            nc.vector.tensor_mul(exc[:], exc[:], mhi[:])
            nc.vector.tensor_sub(g0[:], g0[:], exc[:])
            nc.vector.tensor_mul(g0[:], g0[:], mlo[:])
            return g0

        def aux_pre(b, inv4, pt_b):
            """softmax-normalized aux output for the block, gate NOT yet
            applied: axp = (attn @ av) / sum  -> bf16 SBUF."""
            axp = aux_p.tile([BLK, H], BF16, tag="axp", name=f"axp{b}")
            for j in range(NHCH):
                ax = ax_ps.tile([BLK, HCH], F32, tag="ax", name=f"ax{b}_{j}")
                nc.tensor.matmul(
                    ax[:], lhsT=pt_b[:], rhs=av[:, j * HCH : (j + 1) * HCH]
                )
                nc.scalar.activation(
                    axp[:, j * HCH : (j + 1) * HCH],
                    ax[:],
                    AF.Copy,
                    scale=inv4[:, b : b + 1],
                )
            return axp

        def finish(b, g0, axp):
            """out = (axp * gate) + pao fused on DVE, in 2 halves, each
            stored as soon as it is ready."""
            out_t = out_p.tile([BLK, H], BF16, tag="out", name=f"out{b}")
            r0 = b * BLK
            HH = H // 2
            for half in range(2):
                sl = slice(half * HH, (half + 1) * HH)
                nc.vector.scalar_tensor_tensor(
                    out=out_t[:, sl],
                    in0=axp[:, sl],
                    scalar=g0[:],
                    in1=pao_tiles[b][:, sl],
                    op0=ALU.mult,
                    op1=ALU.add,
                )
                nc.gpsimd.dma_start(
                    out=out_d[r0 : r0 + BLK, sl], in_=out_t[:, sl]
                )

        # ================= issue order =================
        # PE: [qproj][scores][b0][aux-pre 0..1][b1][aux-pre 2..3][b2][b3]

        # ---- q projection: qT[64, 512] (fp8, host-prescaled by WSC) ----
        qt_full = mi_ps.tile([BLK, HCH], F32, tag="mi", name="qt_psum")
        qt_psum = qt_full[:D, :ROWS]
        for k in range(KT):
            nc.tensor.matmul(
                qt_psum[:],
                lhsT=wqt[:, k * D : (k + 1) * D],
                rhs=hst_t[:, k * ROWS : (k + 1) * ROWS],
                start=(k == 0),
                stop=(k == KT - 1),
            )
        qt_sb = const_p.tile([D, ROWS], BF16, tag="qt_sb")
        nc.scalar.activation(qt_sb[:], qt_psum[:], AF.Copy, scale=1.0 / WSC)

        # ---- scores / softmax numerator / attn transpose, all blocks ----
        inv4 = const_p.tile([128, NBLK], F32, tag="inv4")
        pt_all = []
        for b in range(NBLK):
            r0 = b * BLK
            sc_full = mi_ps.tile([BLK, HCH], F32, tag="mi", name=f"sc_psum{b}")
            sc_psum = sc_full[:, :NS]
            nc.tensor.matmul(sc_psum[:], lhsT=qt_sb[:, r0 : r0 + BLK], rhs=akt[:])
            sc_sb = small_p.tile([BLK, NS], F32, tag="sc_sb", name=f"scb{b}")
            nc.vector.tensor_add(sc_sb[:], sc_psum[:], cst[:, 4 : 4 + NS])
            p_t = small_p.tile([BLK, NS], F32, tag="p", name=f"p{b}")
            ssum = small_p.tile([BLK, 1], F32, tag="ssum", name=f"ss{b}")
            nc.scalar.activation(
                p_t[:], sc_sb[:], AF.Exp, bias=cst[:, 3:4], accum_out=ssum[:]
            )
            nc.vector.reciprocal(inv4[:, b : b + 1], ssum[:])
            pt_full = mi_ps.tile([BLK, HCH], F32, tag="mi", name=f"pt_psum{b}")
            pt_psum = pt_full[:NS, :BLK]
            nc.tensor.transpose(pt_psum[:], p_t[:], ident[:])
            ptb = const_p.tile([NS, BLK], BF16, tag=f"pt{b}")
            nc.scalar.copy(ptb[:], pt_psum[:])
            pt_all.append(ptb)

        acc0 = head_sum(0)
        axp0 = aux_pre(0, inv4, pt_all[0])
        axp1 = aux_pre(1, inv4, pt_all[1])
        g0_0 = entropy_gate(0, acc0)
        finish(0, g0_0, axp0)

        acc1 = head_sum(1)
        axp2 = aux_pre(2, inv4, pt_all[2])
        axp3 = aux_pre(3, inv4, pt_all[3])
        g0_1 = entropy_gate(1, acc1)
        finish(1, g0_1, axp1)

        acc2 = head_sum(2)
        g0_2 = entropy_gate(2, acc2)
        finish(2, g0_2, axp2)

        acc3 = head_sum(3)
        g0_3 = entropy_gate(3, acc3)
        finish(3, g0_3, axp3)

    nc.compile()
    return nc


def _get_graph():
    key = "g"
    if key not in _GRAPH_CACHE:
        _GRAPH_CACHE[key] = build_graph()
    return _GRAPH_CACHE[key]


def _make_in_maps(inputs):
    hs = np.asarray(inputs["hidden_states"], dtype=np.float32).reshape(B * S, H)
    pao = np.asarray(inputs["primary_attention_output"], dtype=np.float32).reshape(
        B * S, H
    )
    paw = np.asarray(inputs["primary_attention_weights"], dtype=np.float32)
    rel = np.asarray(inputs["reliability"], dtype=np.float32)
    wq = np.asarray(inputs["W_q"], dtype=np.float32)
    ak = np.asarray(inputs["aux_keys"], dtype=np.float32)
    av = np.asarray(inputs["aux_values"], dtype=np.float32)
    w1 = float(np.asarray(inputs["gate_w1"]))
    gb = float(np.asarray(inputs["gate_bias"]))

    bf = ml_dtypes.bfloat16
    f8 = ml_dtypes.float8_e4m3
    # W_q.T with the 1/sqrt(64) folded in and a fp8-range prescale of WSC
    # (taken back out when draining the q-projection PSUM), laid out as 32
    # stacked [128, 64] k-tiles along the free axis.
    wqt = (
        (wq * (0.125 * WSC)).T
        .reshape(KT, 128, D)
        .transpose(1, 0, 2)
        .reshape(128, KT * D)
    )
    wqt = np.ascontiguousarray(wqt).astype(f8)
    akt = np.ascontiguousarray(ak.T).astype(bf)
    avc = np.ascontiguousarray(av).astype(bf)

    cst = np.zeros((128, 4 + NS), dtype=np.float32)
    cst[:, 0] = -w1 / RSC    # Sigmoid scale (ent = -r'/RSC)
    cst[:, 1] = gb           # Sigmoid bias
    cst[:, 2] = 1e-10        # Ln bias
    cst[:, 3] = 0.0          # Exp bias (scores)
    cst[:, 4:] = np.log(rel + 1e-10)[None, :]

    paw8 = (paw * PSCALE).astype(f8)
    # doubled identity for the DoubleRow pair-sum: [p, i, m] = eye(p, m)
    eye8 = np.eye(128).astype(f8)
    id8 = np.ascontiguousarray(np.hstack([eye8, eye8]))

    in_maps = []
    for c in range(NCORES):
        bb = c // (NCORES // B)
        s0 = (c % (NCORES // B)) * ROWS
        rows = slice(c * ROWS, (c + 1) * ROWS)
        hst = (
            np.ascontiguousarray(hs[rows].T)
            .astype(f8)
            .reshape(KT, 128, ROWS)
            .transpose(1, 0, 2)
            .reshape(128, KT * ROWS)
        )
        # paw -> [block, group, row, head, s] contiguous per (block, group)
        pw = (
            paw8[bb, :, s0 : s0 + ROWS, :]
            .reshape(NGRP, GH, NBLK, BLK, S)
            .transpose(2, 0, 3, 1, 4)
            .reshape(NBLK * NGRP, BLK, GH * S)
        )
        in_maps.append(
            {
                "hst": np.ascontiguousarray(hst),
                "pao": np.ascontiguousarray(pao[rows]).astype(bf),
                "paw": np.ascontiguousarray(pw),
                "wqt": wqt,
                "akt": akt,
                "av": avc,
                "cst": cst,
                "idt": np.eye(128, dtype=np.float32),
                "id8": id8,
            }
        )
    return in_maps


def kernel(**inputs) -> np.ndarray:
    nc = _get_graph()
    in_maps = _make_in_maps(inputs)
    res = run_bass_kernel_spmd(nc, in_maps, list(range(NCORES)))
    out = np.concatenate([res.results[i]["out"] for i in range(NCORES)], axis=0)
    return np.ascontiguousarray(out.reshape(B, S, H).astype(np.float32))


def kernel_traced(inputs, **kw):
    """test-harness entry: returns (output, BassKernelResults)."""
    nc = _get_graph()
    in_maps = _make_in_maps(inputs)
    res = run_bass_kernel_spmd(nc, in_maps, list(range(NCORES)), trace=True, **kw)
    out = np.concatenate([res.results[i]["out"] for i in range(NCORES)], axis=0)
    return np.ascontiguousarray(out.reshape(B, S, H).astype(np.float32), dtype=np.float32), res
